# revision 10
# baseline (speedup 1.0000x reference)
"""GRU kernel for Trainium2 (8 NeuronCores, data-parallel over batch).

Problem: nn_GRU — X [256, 512, 128] f32, W_z/W_r/W_c [256, 384], b_* [256].
Output: h_history [512, 256, 256] f32.

Sharding: batch 256 -> 8 cores x 32. Each core runs an independent GRU
recurrence over its batch shard; weights are replicated. No collectives.

Per-core layout:
  - h state lives as [h_low(128 partitions), (hc(2), b(32))] columns so the
    recurrent matmuls are lhsT=W.T (stationary weights, K=h features on
    partitions), rhs=h slices, out=[h_out_low, b] in PSUM.
  - Input projections x_t @ W_*x.T + b_* are hoisted out of the recurrence
    and computed per 64-step chunk as wide matmuls over X.T, stored in SBUF
    as xP[gate][hc][col = t*32 + b].
  - h history is written in-place per step into h_hist[:, (s+1)*64:...],
    then bulk PE-transposed at chunk end to [b, h] order for contiguous
    output DMA.
"""

import sys
from contextlib import ExitStack

sys.path.insert(0, "/opt/trn_rl_repo")

import numpy as np

import concourse.bass as bass
import concourse.mybir as mybir
import concourse.tile as tile
from concourse import bacc
from concourse.bass_utils import run_bass_kernel_spmd
from concourse.masks import make_identity

F32 = mybir.dt.float32
AF = mybir.ActivationFunctionType

N_CORES = 8
B = 32          # batch per core
S = 512         # sequence length
I = 128         # input features
H = 256         # hidden features
TC = 64         # timesteps per chunk
NCHUNK = S // TC
P = 128

_CACHED_NC = None


def _build_nc():
    nc = bacc.Bacc(
        "TRN2",
        target_bir_lowering=False,
        debug=False,
        enable_asserts=False,
        num_devices=N_CORES,
    )

    X = nc.dram_tensor("X", [B, S, I], F32, kind="ExternalInput").ap()
    Ws = [
        nc.dram_tensor(n, [H, H + I], F32, kind="ExternalInput").ap()
        for n in ("W_z", "W_r", "W_c")
    ]
    bs = [
        nc.dram_tensor(n, [H], F32, kind="ExternalInput").ap()
        for n in ("b_z", "b_r", "b_c")
    ]
    Y = nc.dram_tensor("Y", [S, B, H], F32, kind="ExternalOutput").ap()

    with tile.TileContext(nc) as tc, ExitStack() as ctx:
        _emit(nc, tc, ctx, X, Ws, bs, Y)

    nc.compile()
    return nc


def _emit(nc, tc, ctx, X, Ws, bs, Y):
    const = ctx.enter_context(tc.tile_pool(name="const", bufs=1))
    wtmp_pool = ctx.enter_context(tc.tile_pool(name="wtmp", bufs=2))
    xpool = ctx.enter_context(tc.tile_pool(name="xn", bufs=3))
    xtpool = ctx.enter_context(tc.tile_pool(name="xt", bufs=2))
    xppool = ctx.enter_context(tc.tile_pool(name="xp", bufs=2))
    hpool = ctx.enter_context(tc.tile_pool(name="hh", bufs=2))
    spool = ctx.enter_context(tc.tile_pool(name="work", bufs=3))
    opool = ctx.enter_context(tc.tile_pool(name="ost", bufs=3))
    ppool_t = ctx.enter_context(tc.tile_pool(name="pt", bufs=2, space="PSUM"))
    ppool_x = ctx.enter_context(tc.tile_pool(name="px", bufs=2, space="PSUM"))
    ppool_zr = ctx.enter_context(tc.tile_pool(name="pzr", bufs=2, space="PSUM"))
    ppool_c = ctx.enter_context(tc.tile_pool(name="pc", bufs=2, space="PSUM"))

    identity = const.tile([P, P], F32, tag="ident")
    make_identity(nc, identity)

    # --- weights: transpose to lhsT layout [k_features(part), m_out] ---
    # WhT[g][m][k] : W_g[m*128:(m+1)*128, k*128:(k+1)*128].T
    # WxT[g][m]    : W_g[m*128:(m+1)*128, 256:384].T
    WhT = [[[None] * 2 for _ in range(2)] for _ in range(3)]
    WxT = [[None] * 2 for _ in range(3)]
    for g in range(3):
        for m in range(2):
            for k in range(3):  # 0,1 = h chunks; 2 = x chunk
                wtmp = wtmp_pool.tile([P, P], F32, tag="wtmp")
                nc.sync.dma_start(
                    wtmp[:], Ws[g][m * P : (m + 1) * P, k * P : (k + 1) * P]
                )
                pt = ppool_t.tile([P, P], F32, tag="pt")
                nc.tensor.transpose(pt, wtmp, identity)
                wl = const.tile([P, P], F32, tag=f"wl_{g}_{m}_{k}")
                nc.scalar.copy(wl, pt)
                if k < 2:
                    WhT[g][m][k] = wl
                else:
                    WxT[g][m] = wl

    # biases as [128, 2] (partition = h_low, col = hc)
    b_sb = []
    for g in range(3):
        bt = const.tile([P, 2], F32, tag=f"b_{g}")
        nc.sync.dma_start(bt[:], bs[g].rearrange("(hc p) -> p hc", p=P))
        b_sb.append(bt)

    prev_tail = None
    for c in range(NCHUNK):
        t0 = c * TC

        # --- X load + transpose: xt[:, j*128 + boff*64 + toff] = X[2j+boff, t0+toff, :] ---
        xt = xtpool.tile([P, 16 * P], F32, tag="xt")
        for j in range(16):
            xn = xpool.tile([P, P], F32, tag="xn")
            for boff in range(2):
                nc.sync.dma_start(
                    xn[boff * TC : (boff + 1) * TC, :],
                    X[2 * j + boff, t0 : t0 + TC, :],
                )
            pt = ppool_t.tile([P, P], F32, tag="pt")
            nc.tensor.transpose(pt, xn, identity)
            nc.vector.tensor_copy(xt[:, j * P : (j + 1) * P], pt)

        # --- input projections for this chunk ---
        # xp_zr[:, grp, t*32 + b] for grp in (z0, z1, r0, r1); xp_c[:, m, t*32 + b]
        xp_zr = xppool.tile([P, 4, TC * B], F32, tag="xpzr")
        xp_c = xppool.tile([P, 2, TC * B], F32, tag="xpc")
        for g in range(3):
            for m in range(2):
                if g < 2:
                    dst = xp_zr[:, g * 2 + m, :]
                else:
                    dst = xp_c[:, m, :]
                # view dst cols (t*32 + b) as [bg, t] slabs of 8 batch rows
                dst_v = dst.rearrange("p (t bg) -> p bg t", t=TC)
                for j0 in range(4):
                    px = ppool_x.tile([P, 512], F32, tag="px")
                    nc.tensor.matmul(
                        px,
                        lhsT=WxT[g][m],
                        rhs=xt[:, j0 * 512 : (j0 + 1) * 512],
                        start=True,
                        stop=True,
                    )
                    # px col = bg_local*64 + t ; dst col = t*32 + (8*j0 + bg_local)
                    nc.scalar.activation(
                        dst_v[:, 8 * j0 : 8 * j0 + 8, :],
                        px.rearrange("p (bg t) -> p bg t", bg=8),
                        AF.Identity,
                        bias=b_sb[g][:, m : m + 1],
                    )

        # --- recurrence ---
        h_hist = hpool.tile([P, (TC + 1) * 64], F32, tag="hh")
        if c == 0:
            nc.vector.memset(h_hist[:, 0:64], 0.0)
        else:
            nc.vector.tensor_copy(h_hist[:, 0:64], prev_tail)

        for s in range(TC):
            hprev = h_hist[:, s * 64 : (s + 1) * 64]
            hnew = h_hist[:, (s + 1) * 64 : (s + 2) * 64]

            pzr = ppool_zr.tile([P, 128], F32, tag="pzr")
            for grp in range(4):  # z0 z1 r0 r1
                g, m = divmod(grp, 2)
                for k in range(2):
                    nc.tensor.matmul(
                        pzr[:, grp * 32 : (grp + 1) * 32],
                        lhsT=WhT[g][m][k],
                        rhs=hprev[:, k * 32 : (k + 1) * 32],
                        start=(k == 0),
                        stop=(k == 1),
                    )
            zr_in = spool.tile([P, 128], F32, tag="zrin")
            nc.vector.tensor_add(
                zr_in.rearrange("p (g b) -> p g b", g=4),
                pzr.rearrange("p (g b) -> p g b", g=4),
                xp_zr[:, :, s * B : (s + 1) * B],
            )
            zr_act = spool.tile([P, 128], F32, tag="zract")
            nc.scalar.activation(zr_act, zr_in, AF.Sigmoid)

            rh = spool.tile([P, 64], F32, tag="rh")
            nc.vector.tensor_mul(rh, zr_act[:, 64:128], hprev)

            pc = ppool_c.tile([P, 64], F32, tag="pc")
            for m in range(2):
                for k in range(2):
                    nc.tensor.matmul(
                        pc[:, m * 32 : (m + 1) * 32],
                        lhsT=WhT[2][m][k],
                        rhs=rh[:, k * 32 : (k + 1) * 32],
                        start=(k == 0),
                        stop=(k == 1),
                    )
            c_in = spool.tile([P, 64], F32, tag="cin")
            nc.vector.tensor_add(
                c_in.rearrange("p (m b) -> p m b", m=2),
                pc.rearrange("p (m b) -> p m b", m=2),
                xp_c[:, :, s * B : (s + 1) * B],
            )
            c_act = spool.tile([P, 64], F32, tag="cact")
            nc.scalar.activation(c_act, c_in, AF.Tanh)

            # h_new = h + z * (c - h)
            d = spool.tile([P, 64], F32, tag="d")
            nc.vector.tensor_sub(d, c_act, hprev)
            e = spool.tile([P, 64], F32, tag="e")
            nc.vector.tensor_mul(e, d, zr_act[:, 0:64])
            nc.vector.tensor_add(hnew, hprev, e)

        prev_tail = h_hist[:, TC * 64 : (TC + 1) * 64]

        # --- output: transpose h_hist -> [b, h] order and DMA out ---
        for k in range(TC // 2):
            pt = ppool_t.tile([P, P], F32, tag="pt")
            nc.tensor.transpose(pt, h_hist[:, 64 + k * P : 64 + (k + 1) * P], identity)
            ost = opool.tile([P, P], F32, tag="ost")
            nc.vector.tensor_copy(ost, pt)
            for t2 in range(2):
                for hc in range(2):
                    nc.sync.dma_start(
                        Y[t0 + 2 * k + t2, :, hc * P : (hc + 1) * P],
                        ost[t2 * 64 + hc * 32 : t2 * 64 + (hc + 1) * 32, :],
                    )


def _get_nc():
    global _CACHED_NC
    if _CACHED_NC is None:
        _CACHED_NC = _build_nc()
    return _CACHED_NC


def _run(inputs, trace=False):
    nc = _get_nc()
    X = np.ascontiguousarray(np.asarray(inputs["X"], dtype=np.float32))
    wnames = ("W_z", "W_r", "W_c")
    bnames = ("b_z", "b_r", "b_c")
    ws = {n: np.ascontiguousarray(np.asarray(inputs[n], dtype=np.float32)) for n in wnames}
    bb = {n: np.ascontiguousarray(np.asarray(inputs[n], dtype=np.float32)) for n in bnames}
    in_maps = []
    for core in range(N_CORES):
        m = {"X": np.ascontiguousarray(X[core * B : (core + 1) * B])}
        m.update(ws)
        m.update(bb)
        in_maps.append(m)
    res = run_bass_kernel_spmd(nc, in_maps, list(range(N_CORES)), trace=trace)
    out = np.concatenate([res.results[c]["Y"] for c in range(N_CORES)], axis=1)
    return out, res


def kernel(**inputs) -> np.ndarray:
    out, _ = _run(inputs, trace=False)
    return out
